# revision 28
# baseline (speedup 1.0000x reference)
"""Trainium2 Bass kernel for nn_CSMAdapter (dense DiT-style transformer).

Sharding: DP-4 over batch, pair-replicated (cores 2b and 2b+1 both compute
batch b's full 512-token forward; zero per-block collectives). Two
start-time collectives, both overlapped with front compute:
  - AllReduce of the CSM w-chain partial (w_prime.T), sharded 8 ways.
  - AllGather of AdaLN/mod modulation vectors, N-sharded 8 ways.

Layout: feature-major activations (x.T [D, T] as [128, T] tiles). All
weights shipped host-side pre-transposed and pre-tiled as SBUF images
[128, ...]: `_img` (k-stacked, rhs use) or `_img_m` (m-outer slabs, lhsT
use) so every weight load is one contiguous DMA. Matmuls in fp32r (full PE
rate at N>=256); FFN second matmul in bf16 (h and W2).
"""
import os
import numpy as np
import ml_dtypes

import concourse.bass as bass
import concourse.bacc as bacc
import concourse.tile as tile
import concourse.mybir as mybir
from concourse.bass_utils import run_bass_kernel_spmd
from concourse.masks import make_identity

F32 = mybir.dt.float32
F32R = mybir.dt.float32r
BF16 = mybir.dt.bfloat16
AF = mybir.ActivationFunctionType
ALU = mybir.AluOpType

D = 1024
DL = 3072
T = 512
HEADS, DH = 8, 64
GROUPS = 16
NMEL = 100
DEPTH = int(os.environ.get("CSM_DEPTH", "8"))
KT = 128
NK_D = D // KT      # 8
NK_DL = DL // KT    # 24
NT = T // KT        # 4

MODW = 9 * D
MSHARD = MODW // 8   # 1152
NCH = 3
MODCH = MSHARD // NCH  # 384

EPS = 1e-5


def _img(wT):
    K, M = wT.shape
    return np.ascontiguousarray(
        wT.reshape(K // KT, KT, M).transpose(1, 0, 2).reshape(KT, (K // KT) * M)
    ).astype(np.float32)


def _img_m(wT, mt=KT):
    K, M = wT.shape
    nk, nm = K // KT, M // mt
    a = wT.reshape(nk, KT, nm, mt).transpose(1, 2, 0, 3)
    return np.ascontiguousarray(a.reshape(KT, nm * nk * mt)).astype(np.float32)


def _col(b):
    return np.ascontiguousarray(np.asarray(b, np.float32).reshape(-1, KT).T)


def build_program():
    nc = bacc.Bacc("TRN2", target_bir_lowering=False, debug=False, num_devices=8)

    def din(name, shape, dt=F32):
        return nc.dram_tensor(name, shape, dt, kind="ExternalInput")

    lt = din("lt", [KT, NK_DL * T])
    wit_m = din("wit_m", [KT, NK_D * NK_DL * KT])
    wit_k = din("wit_k", [KT, NK_DL * D])
    ibias = din("ibias", [KT, NK_D])
    wd_sl = din("wd_sl", [KT, NK_DL * KT])
    p_m = din("p_m", [KT, NK_D * NK_D * KT])
    p_k = din("p_k", [KT, NK_D * D])
    pt_m = din("pt_m", [KT, NK_D * NK_D * KT])
    p_myrows = din("p_myrows", [KT, D])
    smt_k = din("smt_k", [KT, NK_D * D])
    sinembT = din("sinembT", [KT, 2 * 4])
    tw1 = din("tw1", [KT, NK_D * 2 * KT])
    tb1 = din("tb1", [KT, NK_D])
    tw2 = din("tw2", [KT, NK_D * NK_D * KT])
    tb2 = din("tb2", [KT, NK_D])
    admod = din("admod", [KT, DEPTH * NK_D * MSHARD])
    admodb = din("admodb", [1, DEPTH * MSHARD])
    sel = din("sel", [KT, 4])
    maskb = din("maskb", [KT, NT])
    c1w = din("c1w", [KT, (GROUPS // 2) * 3 * 64])
    c1b = din("c1b", [KT, NK_D])
    c2w = din("c2w", [KT, (GROUPS // 2) * 3 * 64])
    c2b = din("c2b", [KT, NK_D])
    wqk = din("wqk", [KT, DEPTH * 8 * NK_D * KT])
    bqk = din("bqk", [KT, DEPTH * 8])
    wv_k = din("wv_k", [KT, DEPTH * NK_D * 512])
    bv_row = din("bv_row", [1, DEPTH * 512])
    wo_m = din("wo_m", [64, DEPTH * NK_D * 8 * KT])
    bo = din("bo", [KT, DEPTH * NK_D])
    w1_m = din("w1_m", [KT, DEPTH * 32 * NK_D * KT])
    fb1 = din("fb1", [KT, DEPTH * 32])
    w2_m = din("w2_m", [KT, DEPTH * NK_D * 32 * KT], BF16)
    fb2 = din("fb2", [KT, DEPTH * NK_D])
    melw = din("melw", [KT, NK_D * NMEL])
    melb = din("melb", [NMEL, 1])
    consts = din("consts", [KT, 258])
    consts_bf = din("consts_bf", [KT, 2], BF16)
    fng = din("fng", [KT, NK_D])
    fnb = din("fnb", [KT, NK_D])

    mel_out = nc.dram_tensor("mel_out", [NMEL, T], F32, kind="ExternalOutput")
    tap_name = os.environ.get("CSM_TAP", "")
    tap_out = None
    if tap_name:
        tap_out = nc.dram_tensor("tap", [KT, NK_D * T], F32, kind="ExternalOutput")

    RG = [list(range(8))]

    with tile.TileContext(nc) as tc, nc.allow_low_precision(reason="fp32r is fp32-width"):
        with (
            tc.tile_pool(name="g", bufs=1) as G,
            tc.tile_pool(name="rows", bufs=4) as ROWS,
            tc.tile_pool(name="dram", bufs=1, space="DRAM") as DRAM,
        ):
            consts_sb = G.tile([KT, 258], F32R, tag="consts_sb")
            nc.sync.dma_start(consts_sb[:], consts[:].bitcast(F32R))
            identt = consts_sb[:, 0:KT]
            ones_col = consts_sb[:, KT:KT + 1]
            zeros_col = consts_sb[:, 129:130]
            ones_m = consts_sb[0:1, 130:258]
            constsbf_sb = G.tile([KT, 2], BF16, tag="constsbf_sb")
            nc.sync.dma_start(constsbf_sb[:], consts_bf[:])
            ones_col_bf = constsbf_sb[:, 0:1]
            selt = G.tile([KT, 4], F32, tag="selt")
            nc.sync.dma_start(selt[:], sel[:])
            maskbt = G.tile([KT, NT], F32, tag="maskbt")
            nc.sync.dma_start(maskbt[:], maskb[:])


            xT = [G.tile([KT, T], F32R, tag=f"x{i}", name=f"x{i}") for i in range(NK_D)]

            ar_in = DRAM.tile([D, D], F32, tag="ar_in")
            ar_out = DRAM.tile([D, D], F32, tag="ar_out")
            ag_in = DRAM.tile([DEPTH * MSHARD, 4], F32, tag="ag_in")
            ag_out = DRAM.tile([8 * DEPTH * MSHARD, 4], F32, tag="ag_out")

            # =========================================================
            # FRONT
            # =========================================================
            with (
                tc.tile_pool(name="f", bufs=1) as F_,
                tc.tile_pool(name="fps", bufs=2, space="PSUM") as FPS,
                tc.tile_pool(name="fps2", bufs=2, space="PSUM") as FPS2,
                tc.tile_pool(name="fpsw", bufs=2, space="PSUM") as FPSW,
            ):
                x1 = F_.tile([KT, NK_D * T], F32R, tag="x1")

                # ---- phase A: temb chain + mod matvecs + AG ----
                with (
                    tc.tile_pool(name="fa", bufs=1) as FA,
                    tc.tile_pool(name="fwa", bufs=2) as FWA,
                ):
                    semb = FA.tile([KT, 2 * 4], F32R, tag="semb")
                    nc.sync.dma_start(semb[:], sinembT[:].bitcast(F32R))
                    tb1t = FA.tile([KT, NK_D], F32, tag="tb1t")
                    nc.sync.dma_start(tb1t[:], tb1[:])
                    tb2t = FA.tile([KT, NK_D], F32, tag="tb2t")
                    nc.sync.dma_start(tb2t[:], tb2[:])
                    u_col = FA.tile([KT, NK_D * 4], F32R, tag="u_col")
                    for m in range(NK_D):
                        wmt = FWA.tile([KT, 2 * KT], F32R, tag="tw1w")
                        nc.sync.dma_start(wmt[:], tw1[:, m * 2 * KT:(m + 1) * 2 * KT].bitcast(F32R))
                        ps = FPS.tile([KT, T], F32, tag="fp")
                        for k in range(2):
                            nc.tensor.matmul(ps[:, 0:4], wmt[:, k * KT:(k + 1) * KT],
                                             semb[:, k * 4:(k + 1) * 4],
                                             start=(k == 0), stop=(k == 1))
                        nc.scalar.activation(u_col[:, m * 4:(m + 1) * 4], ps[:, 0:4],
                                             AF.Silu, bias=tb1t[:, m:m + 1])
                    s_col = FA.tile([KT, NK_D * 4], F32R, tag="s_col")
                    for m in range(NK_D):
                        wmt = FWA.tile([KT, NK_D * KT], F32R, tag="tw2w")
                        nc.sync.dma_start(wmt[:], tw2[:, m * NK_D * KT:(m + 1) * NK_D * KT].bitcast(F32R))
                        ps = FPS.tile([KT, T], F32, tag="fp")
                        for k in range(NK_D):
                            nc.tensor.matmul(ps[:, 0:4], wmt[:, k * KT:(k + 1) * KT],
                                             u_col[:, k * 4:(k + 1) * 4],
                                             start=(k == 0), stop=(k == NK_D - 1))
                        tmb = FA.tile([KT, 4], F32, tag="tmb")
                        nc.scalar.activation(tmb[:], ps[:, 0:4], AF.Identity,
                                             bias=tb2t[:, m:m + 1])
                        nc.scalar.activation(s_col[:, m * 4:(m + 1) * 4], tmb[:], AF.Silu)

                    ag_in_v = ag_in[:].rearrange("(k p cg cc) b -> k cg cc b p",
                                                 k=DEPTH, p=KT, cg=NCH, cc=NCH)
                    for blk in range(DEPTH):
                        for nch in range(NCH):
                            col0 = blk * NK_D * MSHARD
                            ps = FPS2.tile([4, MODCH], F32, tag="fp2")
                            for k in range(NK_D):
                                wsl = FWA.tile([KT, MODCH], F32R, tag="admw")
                                nc.sync.dma_start(
                                    wsl[:],
                                    admod[:, col0 + k * MSHARD + nch * MODCH:
                                          col0 + k * MSHARD + (nch + 1) * MODCH].bitcast(F32R))
                                nc.tensor.matmul(ps[:], s_col[:, k * 4:(k + 1) * 4],
                                                 wsl[:], start=(k == 0), stop=False)
                            adb = FWA.tile([1, MODCH], F32R, tag="adb")
                            nc.sync.dma_start(
                                adb[:], admodb[:, blk * MSHARD + nch * MODCH:
                                               blk * MSHARD + (nch + 1) * MODCH].bitcast(F32R))
                            nc.tensor.matmul(ps[:], ones_m[:, 0:4], adb[:],
                                             start=False, stop=True)
                            msb = FA.tile([4, MODCH], F32, tag="msb")
                            nc.vector.tensor_copy(msb[:], ps[:])
                            for cc in range(NCH):
                                nc.sync.dma_start(
                                    ag_in_v[blk, nch, cc],
                                    msb[:, cc * KT:(cc + 1) * KT])
                    nc.gpsimd.collective_compute(
                        "AllGather", ALU.bypass, replica_groups=RG,
                        ins=[ag_in.opt()], outs=[ag_out.opt()])

                # ---- phase B: w-chain shard + AR ----
                with (
                    tc.tile_pool(name="fb", bufs=1) as FB,
                    tc.tile_pool(name="fwb", bufs=2) as FWB,
                ):
                    wp0 = FPSW.tile([KT, T], F32, tag="wp")
                    wp1 = FPSW.tile([KT, T], F32, tag="wp")
                    for k in range(NK_DL):
                        wik = FWB.tile([KT, D], F32R, tag="wik")
                        nc.sync.dma_start(wik[:], wit_k[:, k * D:(k + 1) * D].bitcast(F32R))
                        wdk = FWB.tile([KT, KT], F32R, tag="wdk")
                        nc.sync.dma_start(wdk[:], wd_sl[:, k * KT:(k + 1) * KT].bitcast(F32R))
                        nc.tensor.matmul(wp0[:], wdk[:],
                                         wik[:, 0:T], start=(k == 0), stop=(k == NK_DL - 1))
                        nc.tensor.matmul(wp1[:], wdk[:],
                                         wik[:, T:D], start=(k == 0), stop=(k == NK_DL - 1))
                    wtp = FB.tile([KT, D], F32R, tag="wtp")
                    nc.vector.tensor_copy(wtp[:, 0:T], wp0[:])
                    nc.vector.tensor_copy(wtp[:, T:D], wp1[:])
                    wtpT = FB.tile([KT, NK_D * KT], F32R, tag="wtpT")
                    for c in range(NK_D):
                        pst = FPS.tile([KT, T], F32, tag="fp")
                        nc.tensor.transpose(pst[:, 0:KT].bitcast(F32R),
                                            wtp[:, c * KT:(c + 1) * KT], identt)
                        nc.vector.tensor_copy(wtpT[:, c * KT:(c + 1) * KT], pst[:, 0:KT])
                    b_r = FB.tile([KT, D], F32R, tag="b_r")
                    for nh in range(2):
                        psb = FPSW.tile([KT, T], F32, tag="wp")
                        for k in range(NK_D):
                            pk = FWB.tile([KT, T], F32R, tag="pkt")
                            nc.sync.dma_start(
                                pk[:], p_k[:, k * D + nh * T:k * D + (nh + 1) * T].bitcast(F32R))
                            nc.tensor.matmul(psb[:], wtpT[:, k * KT:(k + 1) * KT], pk[:],
                                             start=(k == 0), stop=(k == NK_D - 1))
                        nc.vector.tensor_copy(b_r[:, nh * T:(nh + 1) * T], psb[:])
                    for m in range(NK_D):
                        for nh in range(2):
                            psw = FPS.tile([KT, T], F32, tag="fp")
                            pmrm = FWB.tile([KT, KT], F32R, tag="wdk", name=f"pmr{m}_{nh}")
                            nc.sync.dma_start(
                                pmrm[:], p_myrows[:, m * KT:(m + 1) * KT].bitcast(F32R))
                            nc.tensor.matmul(psw[:], pmrm[:],
                                             b_r[:, nh * T:(nh + 1) * T],
                                             start=True, stop=True)
                            wsb = FWB.tile([KT, T], F32, tag="wprt")
                            nc.vector.tensor_copy(wsb[:], psw[:])
                            nc.sync.dma_start(
                                ar_in[m * KT:(m + 1) * KT, nh * T:(nh + 1) * T], wsb[:])
                    nc.gpsimd.collective_compute(
                        "AllReduce", ALU.add, replica_groups=RG,
                        ins=[ar_in.opt()], outs=[ar_out.opt()])

                # ---- phase C: input projection ----
                with (
                    tc.tile_pool(name="fc", bufs=1) as FC,
                    tc.tile_pool(name="fwc", bufs=2) as FWC,
                ):
                    lt_sb = FC.tile([KT, NK_DL * T], F32R, tag="lt_sb")
                    nc.sync.dma_start(lt_sb[:], lt[:].bitcast(F32R))
                    ibias_sb = FC.tile([KT, NK_D], F32, tag="ibias_sb")
                    nc.sync.dma_start(ibias_sb[:], ibias[:])
                    for m in range(NK_D):
                        wmt = FWC.tile([KT, NK_DL * KT], F32R, tag="wim")
                        nc.sync.dma_start(
                            wmt[:], wit_m[:, m * NK_DL * KT:(m + 1) * NK_DL * KT].bitcast(F32R))
                        ps = FPS.tile([KT, T], F32, tag="fp")
                        for k in range(NK_DL):
                            nc.tensor.matmul(ps[:], wmt[:, k * KT:(k + 1) * KT],
                                             lt_sb[:, k * T:(k + 1) * T],
                                             start=(k == 0), stop=(k == NK_DL - 1))
                        nc.scalar.activation(x1[:, m * T:(m + 1) * T], ps[:],
                                             AF.Identity, bias=ibias_sb[:, m:m + 1])

                # ---- phase D: mwT + einsum ----
                with (
                    tc.tile_pool(name="fd", bufs=1) as FD,
                    tc.tile_pool(name="fwd", bufs=2) as FWD,
                ):
                    mwT = FD.tile([KT, NK_D * D], F32R, tag="mwT")
                    for k in range(NK_D):
                        wpr = FWD.tile([KT, D], F32, tag="wpr")
                        nc.sync.dma_start(wpr[:], ar_out[k * KT:(k + 1) * KT, :])
                        sms = FWD.tile([KT, D], F32, tag="sms")
                        nc.sync.dma_start(sms[:], smt_k[:, k * D:(k + 1) * D])
                        sig = FWD.tile([KT, D], F32, tag="sig")
                        nc.scalar.activation(sig[:], sms[:], AF.Sigmoid)
                        nc.vector.tensor_tensor(mwT[:, k * D:(k + 1) * D],
                                                wpr[:], sig[:], ALU.mult)
                    z1 = FD.tile([KT, NK_D * T], F32R, tag="z1")
                    for m in range(NK_D):
                        wmt = FWD.tile([KT, NK_D * KT], F32R, tag="pmw")
                        nc.sync.dma_start(
                            wmt[:], p_m[:, m * NK_D * KT:(m + 1) * NK_D * KT].bitcast(F32R))
                        ps = FPS.tile([KT, T], F32, tag="fp")
                        for k in range(NK_D):
                            nc.tensor.matmul(ps[:], wmt[:, k * KT:(k + 1) * KT],
                                             x1[:, k * T:(k + 1) * T],
                                             start=(k == 0), stop=(k == NK_D - 1))
                        nc.vector.tensor_copy(z1[:, m * T:(m + 1) * T], ps[:])
                    z2 = F_.tile([KT, NK_D * T], F32R, tag="x1", name="z2")
                    for m in range(NK_D):
                        ps = FPS.tile([KT, T], F32, tag="fp")
                        for k in range(NK_D):
                            nc.tensor.matmul(ps[:], mwT[:, k * D + m * KT:k * D + (m + 1) * KT],
                                             z1[:, k * T:(k + 1) * T],
                                             start=(k == 0), stop=(k == NK_D - 1))
                        nc.vector.tensor_copy(z2[:, m * T:(m + 1) * T], ps[:])

                # ---- phase E: P@z2 + conv position embedding ----
                with (
                    tc.tile_pool(name="fe", bufs=1) as FE,
                    tc.tile_pool(name="fwe", bufs=2) as FWE,
                ):
                    xp1 = [FE.tile([KT, T + 2], F32R, tag=f"xp1_{i}", name=f"xp1_{i}")
                           for i in range(NK_D)]
                    xp2 = [FE.tile([KT, T + 2], F32R, tag=f"xp2_{i}", name=f"xp2_{i}")
                           for i in range(NK_D)]
                    for i in range(NK_D):
                        nc.vector.tensor_copy(xp1[i][:, 0:1], zeros_col)
                        nc.vector.tensor_copy(xp1[i][:, T + 1:T + 2], zeros_col)
                        nc.vector.tensor_copy(xp2[i][:, 0:1], zeros_col)
                        nc.vector.tensor_copy(xp2[i][:, T + 1:T + 2], zeros_col)
                    for m in range(NK_D):
                        wmt = FWE.tile([KT, NK_D * KT], F32R, tag="ptw")
                        nc.sync.dma_start(
                            wmt[:], pt_m[:, m * NK_D * KT:(m + 1) * NK_D * KT].bitcast(F32R))
                        ps = FPS.tile([KT, T], F32, tag="fp")
                        for k in range(NK_D):
                            nc.tensor.matmul(ps[:], wmt[:, k * KT:(k + 1) * KT],
                                             z2[:, k * T:(k + 1) * T],
                                             start=(k == 0), stop=(k == NK_D - 1))
                        nc.vector.tensor_copy(xp1[m][:, 1:T + 1], ps[:])
                    c1w_sb = FWE.tile([KT, (GROUPS // 2) * 3 * 64], F32R, tag="cw", name="c1w_sb")
                    nc.sync.dma_start(c1w_sb[:], c1w[:].bitcast(F32R))
                    c2w_sb = FWE.tile([KT, (GROUPS // 2) * 3 * 64], F32R, tag="cw", name="c2w_sb")
                    nc.sync.dma_start(c2w_sb[:], c2w[:].bitcast(F32R))
                    c1b_sb = FE.tile([KT, NK_D], F32, tag="c1b_sb")
                    nc.sync.dma_start(c1b_sb[:], c1b[:])
                    c2b_sb = FE.tile([KT, NK_D], F32, tag="c2b_sb")
                    nc.sync.dma_start(c2b_sb[:], c2b[:])
                    for g in range(GROUPS):
                        ti, ro = g // 2, 64 * (g % 2)
                        ps = FPS2.tile([64, T], F32, tag="fp2")
                        for k in range(3):
                            nc.tensor.matmul(
                                ps[:], c1w_sb[ro:ro + 64, (ti * 3 + k) * 64:(ti * 3 + k + 1) * 64],
                                xp1[ti][ro:ro + 64, k:k + T],
                                start=(k == 0), stop=(k == 2))
                        nc.scalar.activation(xp2[ti][ro:ro + 64, 1:T + 1], ps[:],
                                             AF.Gelu_apprx_tanh,
                                             bias=c1b_sb[ro:ro + 64, ti:ti + 1])
                    for g in range(GROUPS):
                        ti, ro = g // 2, 64 * (g % 2)
                        ps = FPS2.tile([64, T], F32, tag="fp2")
                        for k in range(3):
                            nc.tensor.matmul(
                                ps[:], c2w_sb[ro:ro + 64, (ti * 3 + k) * 64:(ti * 3 + k + 1) * 64],
                                xp2[ti][ro:ro + 64, k:k + T],
                                start=(k == 0), stop=(k == 2))
                        nc.scalar.activation(xT[ti][ro:ro + 64, :], ps[:],
                                             AF.Identity,
                                             bias=c2b_sb[ro:ro + 64, ti:ti + 1])

            # =========================================================
            # BLOCKS
            # =========================================================
            with (
                tc.tile_pool(name="b", bufs=1) as B_,
                tc.tile_pool(name="bw", bufs=3) as BW,
                tc.tile_pool(name="bw2", bufs=2) as BW2,
                tc.tile_pool(name="sq", bufs=2) as SQ,
                tc.tile_pool(name="pmm", bufs=2, space="PSUM") as PMM,
                tc.tile_pool(name="psc", bufs=2, space="PSUM") as PSC,
                tc.tile_pool(name="pst", bufs=2, space="PSUM") as PST,
                tc.tile_pool(name="pbc", bufs=2, space="PSUM") as PBC,
            ):
                ag_out_v = ag_out[:].rearrange("(r k p c) b -> k r b p c",
                                               r=8, k=DEPTH, p=KT, c=9)

                def layernorm(src, dst, scol, bcol):
                    ps_sum = PST.tile([1, T], F32, tag="st")
                    for i in range(NK_D):
                        nc.tensor.matmul(ps_sum[:], ones_col, src[i][:],
                                         start=(i == 0), stop=(i == NK_D - 1))
                    ps_sq = PST.tile([1, T], F32, tag="st")
                    for i in range(NK_D):
                        sqt = SQ.tile([KT, T], F32R, tag="sqt")
                        nc.scalar.activation(sqt[:], src[i][:], AF.Square)
                        nc.tensor.matmul(ps_sq[:], ones_col, sqt[:],
                                         start=(i == 0), stop=(i == NK_D - 1))
                    mean_r = ROWS.tile([1, T], F32, tag="r")
                    nc.vector.tensor_scalar_mul(mean_r[:], ps_sum[:], 1.0 / D)
                    msq_r = ROWS.tile([1, T], F32, tag="r")
                    nc.vector.tensor_tensor(msq_r[:], mean_r[:], mean_r[:], ALU.mult)
                    var_r = ROWS.tile([1, T], F32, tag="r")
                    nc.vector.scalar_tensor_tensor(var_r[:], ps_sq[:], 1.0 / D,
                                                   msq_r[:], ALU.mult, ALU.subtract)
                    nc.vector.tensor_scalar_add(var_r[:], var_r[:], EPS)
                    std_r = ROWS.tile([1, T], F32, tag="r")
                    nc.scalar.activation(std_r[:], var_r[:], AF.Sqrt)
                    rstd_r = ROWS.tile([1, T], F32R, tag="r")
                    nc.vector.reciprocal(rstd_r[:], std_r[:])
                    nb_r = ROWS.tile([1, T], F32R, tag="r")
                    nc.vector.scalar_tensor_tensor(nb_r[:], mean_r[:], -1.0,
                                                   rstd_r[:], ALU.mult, ALU.mult)
                    ps_a = PBC.tile([KT, T], F32, tag="bc")
                    nc.tensor.matmul(ps_a[:], ones_m, rstd_r[:],
                                     start=True, stop=True)
                    ps_b = PBC.tile([KT, T], F32, tag="bc")
                    nc.tensor.matmul(ps_b[:], ones_m, nb_r[:],
                                     start=True, stop=True)
                    for i in range(NK_D):
                        t1 = SQ.tile([KT, T], F32R, tag="lnt")
                        nc.vector.tensor_tensor(t1[:], src[i][:],
                                                ps_a[:].bitcast(F32R), ALU.mult)
                        nc.vector.tensor_tensor(t1[:], t1[:],
                                                ps_b[:].bitcast(F32R), ALU.add)
                        nc.scalar.activation(dst[i][:], t1[:], AF.Identity,
                                             bias=bcol(i), scale=scol(i))

                xln = [B_.tile([KT, T], F32R, tag=f"xln{i}", name=f"xln{i}") for i in range(NK_D)]
                h = [B_.tile([KT, T], BF16, tag=f"h{i}", name=f"h{i}") for i in range(32)]
                qT = [B_.tile([KT, T], F32R, tag=f"qT{i}", name=f"qT{i}") for i in range(4)]
                kTt = [B_.tile([KT, T], F32R, tag=f"kT{i}", name=f"kT{i}") for i in range(4)]
                vt = [B_.tile([KT, 512], BF16, tag=f"v{i}", name=f"v{i}") for i in range(NT)]
                wvt = [B_.tile([KT, 512], F32R, tag=f"wv{i}", name=f"wv{i}") for i in range(NK_D)]
                pxp = [B_.tile([KT, T], BF16, tag=f"px{i}", name=f"px{i}") for i in range(NT)]
                oT = [B_.tile([64, T], F32R, tag=f"oT{i}", name=f"oT{i}") for i in range(HEADS)]

                for blk in range(DEPTH):
                    # ---- modulation columns ----
                    mod4 = SQ.tile([KT, 4 * 72], F32, tag="lnt", name="mod4")
                    for b4 in range(4):
                        for rr in range(8):
                            nc.sync.dma_start(
                                mod4[:, b4 * 72 + rr * 9:b4 * 72 + (rr + 1) * 9],
                                ag_out_v[blk, rr, b4])
                    modc = B_.tile([KT, 72], F32, tag="modc")
                    nc.vector.memset(modc[:], 0.0)
                    for b4 in range(4):
                        nc.vector.scalar_tensor_tensor(
                            modc[:], mod4[:, b4 * 72:(b4 + 1) * 72],
                            selt[:, b4:b4 + 1], modc[:], ALU.mult, ALU.add)
                    for c0 in (8, 32, 56):
                        nc.vector.tensor_scalar_add(modc[:, c0:c0 + 8],
                                                    modc[:, c0:c0 + 8], 1.0)

                    def mc(c):
                        return modc[:, c:c + 1]

                    # per-block bias tiles
                    bqk_b = BW.tile([KT, 8], F32, tag="bqk_b")
                    nc.sync.dma_start(bqk_b[:], bqk[:, blk * 8:(blk + 1) * 8])
                    bo_b = BW.tile([KT, NK_D], F32, tag="bo_b")
                    nc.sync.dma_start(bo_b[:], bo[:, blk * NK_D:(blk + 1) * NK_D])
                    fb1_b = BW.tile([KT, 32], F32, tag="fb1_b")
                    nc.sync.dma_start(fb1_b[:], fb1[:, blk * 32:(blk + 1) * 32])
                    fb2_b = BW.tile([KT, NK_D], F32, tag="fb2_b")
                    nc.sync.dma_start(fb2_b[:], fb2[:, blk * NK_D:(blk + 1) * NK_D])

                    # ---- adaLN pre (in place) ----
                    layernorm(xT, xT, lambda i: mc(8 + i), lambda i: mc(0 + i))
                    # ---- attn LN ----
                    layernorm(xT, xln, lambda i: mc(32 + i), lambda i: mc(24 + i))

                    # ---- attention ----
                    for mi in range(8):
                        dstt = qT[mi] if mi < 4 else kTt[mi - 4]
                        wmt = BW.tile([KT, NK_D * KT], F32R, tag="wqkm")
                        base = blk * 8 * NK_D * KT
                        nc.sync.dma_start(
                            wmt[:], wqk[:, base + mi * NK_D * KT:
                                        base + (mi + 1) * NK_D * KT].bitcast(F32R))
                        ps = PMM.tile([KT, T], F32, tag="mm")
                        for k in range(NK_D):
                            nc.tensor.matmul(ps[:], wmt[:, k * KT:(k + 1) * KT],
                                             xln[k][:],
                                             start=(k == 0), stop=(k == NK_D - 1))
                        nc.scalar.activation(dstt[:], ps[:], AF.Identity,
                                             bias=bqk_b[:, mi:mi + 1])
                    bv_b = BW.tile([1, 512], F32R, tag="bv_b")
                    nc.sync.dma_start(bv_b[:],
                                      bv_row[:, blk * 512:(blk + 1) * 512].bitcast(F32R))
                    for k in range(NK_D):
                        nc.sync.dma_start(
                            wvt[k][:], wv_k[:, blk * NK_D * 512 + k * 512:
                                            blk * NK_D * 512 + (k + 1) * 512].bitcast(F32R))
                    for mt in range(NT):
                        ps = PMM.tile([KT, 512], F32, tag="mm")
                        for k in range(NK_D):
                            nc.tensor.matmul(ps[:], xln[k][:, mt * KT:(mt + 1) * KT],
                                             wvt[k][:], start=(k == 0), stop=False)
                        nc.tensor.matmul(ps[:], ones_m, bv_b[:],
                                         start=False, stop=True)
                        nc.vector.tensor_copy(vt[mt][:], ps[:])
                    for hd in range(HEADS):
                        kt_tile, kro = kTt[hd // 2], 64 * (hd % 2)
                        for kc in range(NT):
                            ps = PSC.tile([KT, T], F32, tag="sc")
                            nc.tensor.matmul(
                                ps[:], kt_tile[kro:kro + 64, kc * KT:(kc + 1) * KT],
                                qT[hd // 2][kro:kro + 64, :], start=True, stop=True)
                            nc.scalar.activation(pxp[kc][:], ps[:], AF.Exp,
                                                 bias=maskbt[:, kc:kc + 1], scale=0.125)
                        ps_ssum = PST.tile([1, T], F32, tag="st")
                        for kc in range(NT):
                            nc.tensor.matmul(ps_ssum[:], ones_col_bf, pxp[kc][:],
                                             start=(kc == 0), stop=(kc == NT - 1))
                        rec_r = ROWS.tile([1, T], F32R, tag="r")
                        nc.vector.reciprocal(rec_r[:], ps_ssum[:])
                        ps_av = PSC.tile([64, T], F32, tag="sc", name=f"av{hd}")
                        for kc in range(NT):
                            nc.tensor.matmul(
                                ps_av[:],
                                vt[kc][:, hd * 64:(hd + 1) * 64], pxp[kc][:],
                                start=(kc == 0), stop=(kc == NT - 1))
                        ps_rep = PBC.tile([64, T], F32, tag="bc", name=f"rep{hd}")
                        nc.tensor.matmul(ps_rep[:], ones_m[:, 0:64],
                                         rec_r[:], start=True, stop=True)
                        rep_sb = SQ.tile([64, T], F32, tag="sqt", name="rep_sb")
                        nc.scalar.copy(rep_sb[:], ps_rep[:])
                        nc.vector.tensor_tensor(oT[hd][:], ps_av[:],
                                                rep_sb[:], ALU.mult)
                    for m in range(NK_D):
                        wmt = BW.tile([64, 8 * KT], F32R, tag="wom")
                        base = blk * NK_D * 8 * KT
                        nc.sync.dma_start(
                            wmt[:], wo_m[:, base + m * 8 * KT:
                                         base + (m + 1) * 8 * KT].bitcast(F32R))
                        ps = PMM.tile([KT, T], F32, tag="mm")
                        for k in range(8):
                            nc.tensor.matmul(ps[:], wmt[:, k * KT:(k + 1) * KT],
                                             oT[k][:], start=(k == 0), stop=(k == 7))
                        yt = SQ.tile([KT, T], F32R, tag="sqt", name="yt")
                        nc.scalar.activation(yt[:], ps[:], AF.Identity,
                                             bias=bo_b[:, m:m + 1])
                        nc.vector.scalar_tensor_tensor(xT[m][:], yt[:], mc(40 + m),
                                                       xT[m][:], ALU.mult, ALU.add)

                    # ---- ffn ----
                    layernorm(xT, xln, lambda i: mc(56 + i), lambda i: mc(48 + i))
                    for m in range(32):
                        wmt = BW.tile([KT, NK_D * KT], F32R, tag="w1m")
                        base = blk * 32 * NK_D * KT
                        nc.sync.dma_start(
                            wmt[:], w1_m[:, base + m * NK_D * KT:
                                         base + (m + 1) * NK_D * KT].bitcast(F32R))
                        ps = PMM.tile([KT, T], F32, tag="mm")
                        for k in range(NK_D):
                            nc.tensor.matmul(ps[:], wmt[:, k * KT:(k + 1) * KT],
                                             xln[k][:], start=(k == 0), stop=(k == NK_D - 1))
                        nc.scalar.activation(h[m][:], ps[:], AF.Gelu_apprx_tanh,
                                             bias=fb1_b[:, m:m + 1])
                    for m in range(NK_D):
                        wmt = BW2.tile([KT, 32 * KT], BF16, tag="w2m")
                        base = blk * NK_D * 32 * KT
                        nc.sync.dma_start(
                            wmt[:], w2_m[:, base + m * 32 * KT:base + (m + 1) * 32 * KT])
                        ps = PMM.tile([KT, T], F32, tag="mm")
                        for k in range(32):
                            nc.tensor.matmul(ps[:], wmt[:, k * KT:(k + 1) * KT],
                                             h[k][:], start=(k == 0), stop=(k == 31))
                        yt = SQ.tile([KT, T], F32R, tag="sqt", name="yt")
                        nc.scalar.activation(yt[:], ps[:], AF.Identity,
                                             bias=fb2_b[:, m:m + 1])
                        nc.vector.scalar_tensor_tensor(xT[m][:], yt[:], mc(64 + m),
                                                       xT[m][:], ALU.mult, ALU.add)

                # ---- final LN + mel ----
                fng_sb = B_.tile([KT, NK_D], F32, tag="fng_sb")
                nc.sync.dma_start(fng_sb[:], fng[:])
                fnb_sb = B_.tile([KT, NK_D], F32, tag="fnb_sb")
                nc.sync.dma_start(fnb_sb[:], fnb[:])
                layernorm(xT, xln, lambda i: fng_sb[:, i:i + 1],
                          lambda i: fnb_sb[:, i:i + 1])
                melw_sb = B_.tile([KT, NK_D * NMEL], F32R, tag="melw_sb")
                nc.sync.dma_start(melw_sb[:], melw[:].bitcast(F32R))
                melb_sb = B_.tile([NMEL, 1], F32, tag="melb_sb")
                nc.sync.dma_start(melb_sb[:], melb[:])
                ps = PMM.tile([KT, T], F32, tag="mm")
                for k in range(NK_D):
                    nc.tensor.matmul(ps[0:NMEL, :], melw_sb[:, k * NMEL:(k + 1) * NMEL],
                                     xln[k][:], start=(k == 0), stop=(k == NK_D - 1))
                melt = B_.tile([NMEL, T], F32, tag="melt")
                nc.scalar.activation(melt[:], ps[0:NMEL, :], AF.Identity, bias=melb_sb[:])
                nc.sync.dma_start(mel_out[:], melt[:])

                if tap_out is not None:
                    src = {"x": xT, "xln": xln, "conv": xT}[tap_name]
                    for i in range(NK_D):
                        nc.sync.dma_start(tap_out[:, i * T:(i + 1) * T],
                                          src[i][:].bitcast(F32))

    nc.compile()
    return nc


# ---------------------------------------------------------------------------
# host wrapper
# ---------------------------------------------------------------------------

_CACHE = {}


def _prep_core(inp, core):
    b, r = core // 2, core
    p = inp["params"]
    blocks = p["blocks"]
    L = inp["llama_embeddings"][b]
    tsteps = inp["timesteps"]
    mask = inp["mask"][b]

    half = 128
    freqs = np.exp(np.arange(half, dtype=np.float32) * (-np.log(10000.0) / (half - 1)))
    ang = tsteps[:, None].astype(np.float32) * 1000.0 * freqs[None, :]
    sinemb = np.concatenate([np.sin(ang), np.cos(ang)], -1).astype(np.float32)

    sel = np.zeros((KT, 4), np.float32)
    sel[:, b] = 1.0
    maskbias = np.where(mask, 0.0, -1e30).astype(np.float32)

    WiT = p["input_proj_w"].T
    WdT = p["proj_down_w"].T
    P = p["P"]

    m = {
        "lt": _img(np.ascontiguousarray(L.T)),
        "wit_m": _img_m(WiT),
        "wit_k": _img(WiT),
        "ibias": _col(p["input_proj_b"]),
        "wd_sl": _img(np.ascontiguousarray(WdT[:, 128 * r:128 * (r + 1)])),
        "p_m": _img_m(P),
        "p_k": _img(P),
        "pt_m": _img_m(np.ascontiguousarray(P.T)),
        "p_myrows": np.ascontiguousarray(P[128 * r:128 * (r + 1), :]).astype(np.float32),
        "smt_k": _img(np.ascontiguousarray(p["spectral_mask"].T)),
        "sinembT": _img(np.ascontiguousarray(sinemb.T)),
        "tw1": _img_m(np.ascontiguousarray(p["t_w1"].T)),
        "tb1": _col(p["t_b1"]),
        "tw2": _img_m(np.ascontiguousarray(p["t_w2"].T)),
        "tb2": _col(p["t_b2"]),
        "sel": sel,
        "maskb": _col(maskbias),
        "melw": _img(np.ascontiguousarray(p["mel_w"].T)),
        "consts_bf": np.concatenate(
            [np.ones((KT, 1)), np.zeros((KT, 1))], axis=1).astype(ml_dtypes.bfloat16),
        "consts": np.concatenate(
            [np.eye(KT, dtype=np.float32),
             np.ones((KT, 1), np.float32),
             np.zeros((KT, 1), np.float32),
             np.ones((KT, KT), np.float32)], axis=1),
        "melb": np.asarray(p["mel_b"], np.float32).reshape(NMEL, 1),
        "fng": _col(p["fn_g"]),
        "fnb": _col(p["fn_b"]),
    }

    def convpack(w):
        out = np.zeros((KT, (GROUPS // 2) * 3 * 64), np.float32)
        for g in range(GROUPS):
            ro = 64 * (g % 2)
            pg = g // 2
            for k in range(3):
                out[ro:ro + 64, (pg * 3 + k) * 64:(pg * 3 + k + 1) * 64] = \
                    w[64 * g:64 * (g + 1), :, k].T
        return out

    m["c1w"] = convpack(p["pc1_w"])
    m["c1b"] = _col(p["pc1_b"])
    m["c2w"] = convpack(p["pc2_w"])
    m["c2b"] = _col(p["pc2_b"])

    admod_l, admodb_l = [], []
    wqk_l, bqk_l, wvk_l, bvr_l, wo_l, bo_l = [], [], [], [], [], []
    w1_l, fb1_l, w2_l, fb2_l = [], [], [], []
    for blk in blocks[:DEPTH]:
        WadT = np.concatenate([blk["adaln_w"], blk["mod_w"]], 0).T
        admod_l.append(_img(np.ascontiguousarray(
            WadT[:, MSHARD * r:MSHARD * (r + 1)])))
        admodb_l.append(np.concatenate([blk["adaln_b"], blk["mod_b"]])
                        [MSHARD * r:MSHARD * (r + 1)])
        wqk_l.append(_img_m(np.ascontiguousarray(
            np.concatenate([blk["wq"].T, blk["wk"].T], 1))))
        bqk_l.append(np.concatenate([blk["bq"], blk["bk"]]))
        wvk_l.append(_img(np.ascontiguousarray(blk["wv"].T)))
        bvr_l.append(blk["bv"])
        WoT = np.ascontiguousarray(blk["wo"].T)  # [512, 1024]
        a = WoT.reshape(8, 64, 8, KT).transpose(1, 2, 0, 3).reshape(64, 8 * 8 * KT)
        wo_l.append(np.ascontiguousarray(a).astype(np.float32))
        bo_l.append(blk["bo"])
        w1_l.append(_img_m(np.ascontiguousarray(blk["ff_w1"].T)))
        fb1_l.append(blk["ff_b1"])
        w2_l.append(_img_m(np.ascontiguousarray(blk["ff_w2"].T)))
        fb2_l.append(blk["ff_b2"])

    m["admod"] = np.concatenate(admod_l, 1) if admod_l else np.zeros((KT, 0), np.float32)
    m["admodb"] = (np.concatenate(admodb_l)[None, :].astype(np.float32)
                   if admodb_l else np.zeros((1, 0), np.float32))
    m["wqk"] = np.concatenate(wqk_l, 1) if wqk_l else np.zeros((KT, 0), np.float32)
    m["bqk"] = (np.concatenate([_col(x) for x in bqk_l], 1)
                if bqk_l else np.zeros((KT, 0), np.float32))
    m["wv_k"] = np.concatenate(wvk_l, 1) if wvk_l else np.zeros((KT, 0), np.float32)
    m["bv_row"] = (np.concatenate(bvr_l)[None, :].astype(np.float32)
                   if bvr_l else np.zeros((1, 0), np.float32))
    m["wo_m"] = np.concatenate(wo_l, 1) if wo_l else np.zeros((KT, 0), np.float32)
    m["bo"] = (np.concatenate([_col(x) for x in bo_l], 1)
               if bo_l else np.zeros((KT, 0), np.float32))
    m["w1_m"] = np.concatenate(w1_l, 1) if w1_l else np.zeros((KT, 0), np.float32)
    m["fb1"] = (np.concatenate([_col(x) for x in fb1_l], 1)
                if fb1_l else np.zeros((KT, 0), np.float32))
    m["w2_m"] = (np.concatenate(w2_l, 1).astype(ml_dtypes.bfloat16)
                 if w2_l else np.zeros((KT, 0), ml_dtypes.bfloat16))
    m["fb2"] = (np.concatenate([_col(x) for x in fb2_l], 1)
                if fb2_l else np.zeros((KT, 0), np.float32))
    return m


def kernel(llama_embeddings, timesteps, mask, params):
    inp = {
        "llama_embeddings": np.asarray(llama_embeddings, np.float32),
        "timesteps": np.asarray(timesteps, np.float32),
        "mask": np.asarray(mask),
        "params": dict(params),
    }
    inp["params"] = {k: (v if k == "blocks" else np.asarray(v, np.float32))
                     for k, v in params.items()}
    inp["params"]["blocks"] = [
        {k: np.asarray(v, np.float32) for k, v in blk.items()}
        for blk in params["blocks"]
    ]

    if "nc" not in _CACHE:
        _CACHE["nc"] = build_program()
    nc = _CACHE["nc"]

    in_maps = [_prep_core(inp, c) for c in range(8)]
    res = run_bass_kernel_spmd(nc, in_maps, list(range(8)))
    out = np.zeros((4, NMEL, T), np.float32)
    for b in range(4):
        out[b] = res.results[2 * b]["mel_out"]
    return out
